# revision 5
# baseline (speedup 1.0000x reference)
"""Conv2d 3x3 via 1-D Winograd F(4,3) along the kh (row) axis.

Half-integer interpolation points {0, +-1, +-1/2, inf} keep every
transform constant an exact power of two in bf16:
  B^T rows (input):  D0 = .25(d0-d2) + (d4-d2)
                     D1 = (d3+d4) - .25(d1+d2)
                     D2 = (d4-d3) + .25(d1-d2)
                     D3 = (d4-d2) - .5(d1-d3)
                     D4 = (d4-d2) + .5(d1-d3)
                     D5 = .25(d1-d3) - (d3-d5)
  A^T rows (output): o0 = m0+m1+m2+m3+m4        o1 = (m1-m2) + .5(m3-m4)
                     o2 = (m1+m2) + .25(m3+m4)  o3 = (m1-m2) + .125(m3-m4) + m5
6 multiplies per 4x1 outputs instead of 12: a 2x TensorEngine FLOP cut
vs direct conv (vs 1.5x for F(2,3)).

Per core (4 images): rows pre-transformed into D_k (k=0..5, quads
q=0..13, both ci-tiles fused per op), weights host-transformed into
U_k[ci,kw,co], kw handled as 3 shifted matmul taps.  Per (image,
co_tile, half of 7 quads): 6 PSUM banks M_k each accumulate 6 matmuls
(2 ci-tiles x 3 kw, K=128, N=392).  Output transform spread over ACT
(PSUM->SBUF copies), DVE (PSUM-reading sums + 4x-mode scalings; u/v
formed from fp32 PSUM so the amplified m3/m4 are not rounded before
they cancel) and Pool (input-transform helpers + bf16 output sums).
Output stored bf16 and widened to fp32 on the host.
"""

import numpy as np
import ml_dtypes

import concourse.bass as bass
import concourse.mybir as mybir
from concourse import bacc
from concourse.tile import TileContext
from concourse.bass_utils import run_bass_kernel_spmd

P = 128
N_CORES = 8
NIMG = 4
CIN = 256
COUT = 256
H = W = 56
HP = WP = 58
CI_T = 2
CO_T = 2
KF = 6                     # winograd row taps
NQ = 14                    # quad rows per image
HQ = 7                     # quads per half
NH = HQ * W                # matmul N per half = 392

_cached = {}


def _build_nc():
    nc = bacc.Bacc("TRN2", target_bir_lowering=False, debug=False,
                   num_devices=N_CORES)

    ip_h = nc.declare_dram_parameter("ip", [NIMG, CIN, HP, WP],
                                     mybir.dt.bfloat16, isOutput=False)
    w_h = nc.declare_dram_parameter("weight", [P, CO_T * CI_T * KF * 3 * P],
                                    mybir.dt.bfloat16, isOutput=False)
    b_h = nc.declare_dram_parameter("bias", [P, CO_T],
                                    mybir.dt.float32, isOutput=False)
    out_h = nc.declare_dram_parameter("out", [NIMG, COUT, H, W],
                                      mybir.dt.bfloat16, isOutput=True)

    ip_v = ip_h.ap().rearrange("n (t p) h w -> n t p h w", p=P)
    w_v = w_h.ap()
    out_v = out_h.ap().rearrange("n (t p) h w -> n t p (h w)", p=P)

    AF = mybir.ActivationFunctionType
    OP = mybir.AluOpType
    BF = mybir.dt.bfloat16
    F32 = mybir.dt.float32

    def woff(ot, it, k, kw):
        return (((ot * CI_T + it) * KF + k) * 3 + kw) * P

    with TileContext(nc) as tc:
        with (
            tc.tile_pool(name="const", bufs=1) as cpool,
            tc.tile_pool(name="pad0", bufs=2) as ppool,
            tc.tile_pool(name="padf", bufs=2) as pfpool,
            tc.tile_pool(name="d0", bufs=12) as d0pool,
            tc.tile_pool(name="df", bufs=12) as dfpool,
            tc.tile_pool(name="tmp", bufs=10) as tpool,
            tc.tile_pool(name="oc", bufs=2) as ocpool,
            tc.tile_pool(name="outs", bufs=4) as opool,
            tc.tile_pool(name="psum", bufs=8, space="PSUM") as pspool,
        ):
            wt = cpool.tile([P, CO_T * CI_T * KF * 3 * P], BF)
            bt = cpool.tile([P, CO_T], F32)

            # --- DMA staging (single HWDGE queue is FIFO: order by first
            # consumption so the startup ramp is short) ---
            WB = KF * 3 * P            # one (ot, it) weight block width
            nc.sync.dma_start(out=wt[:, 0:WB], in_=w_v[:, 0:WB])  # ot0 it0
            pt = [None, None]          # image-0 half tiles [P, 2, 32, WP]
            pt[0] = ppool.tile([P, CI_T, 32, WP], BF, tag="p0", name="pt_h0")
            nc.sync.dma_start(out=pt[0][:, 0, 0:30, :], in_=ip_v[0, 0, :, 0:30])
            nc.sync.dma_start(out=wt[:, WB:2 * WB],
                              in_=w_v[:, WB:2 * WB])              # ot0 it1
            nc.sync.dma_start(out=pt[0][:, 1, 0:30, :], in_=ip_v[0, 1, :, 0:30])
            nc.sync.dma_start(out=bt[:], in_=b_h.ap())
            pt[1] = ppool.tile([P, CI_T, 32, WP], BF, tag="p0", name="pt_h1")
            nc.sync.dma_start(out=pt[1][:, 0, 0:30, :], in_=ip_v[0, 0, :, 28:58])
            nc.sync.dma_start(out=pt[1][:, 1, 0:30, :], in_=ip_v[0, 1, :, 28:58])
            nc.sync.dma_start(out=wt[:, 2 * WB:], in_=w_v[:, 2 * WB:])  # ot1
            fulls = [None] * NIMG
            for n in range(1, NIMG):
                pf = pfpool.tile([P, CI_T, 60, WP], BF, tag="pf",
                                 name=f"pf_{n}")
                for it in range(CI_T):
                    nc.sync.dma_start(out=pf[:, it, 0:HP, :], in_=ip_v[n, it])
                fulls[n] = pf

            def transform(ptile, nq, dpool, dtag, nm, use_pool=True):
                """Row transform, both ci-tiles fused: ptile
                [P, 2, 4*(nq+1), WP] holding rows 4q+a; returns 6 D_k
                tiles [P, 2, nq, WP]."""
                v = ptile.rearrange("p t (q a) c -> p t q a c", a=4)
                d = [v[:, :, 0:nq, a, :] for a in range(4)]
                d4 = v[:, :, 1:nq + 1, 0, :]
                d5 = v[:, :, 1:nq + 1, 1, :]

                def tl(r):
                    return tpool.tile([P, CI_T, nq, WP], BF, tag="tt",
                                      name=f"t_{nm}_{r}")
                # slow Pool ops first so they overlap the DVE/ACT chain;
                # image-0's first transforms stay off Pool (startup path)
                eng = nc.gpsimd if use_pool else nc.vector
                c_ = tl("c"); eng.tensor_tensor(c_[:], d4, d[2], OP.subtract)
                dd = tl("dd"); eng.tensor_tensor(dd[:], d[1], d[3], OP.subtract)
                p_ = tl("p"); eng.tensor_tensor(p_[:], d[3], d4, OP.add)
                q2 = tl("q"); eng.tensor_tensor(q2[:], d4, d[3], OP.subtract)
                a2 = tl("a"); nc.vector.tensor_tensor(a2[:], d[1], d[2], OP.add)
                b2 = tl("b"); nc.vector.tensor_tensor(b2[:], d[1], d[2], OP.subtract)
                e2 = tl("e"); nc.vector.tensor_tensor(e2[:], d[0], d[2], OP.subtract)
                f2 = tl("f"); nc.vector.tensor_tensor(f2[:], d[3], d5, OP.subtract)
                t1 = tl("t1"); nc.scalar.activation(t1[:], a2[:], AF.Identity, scale=0.25)
                t2 = tl("t2"); nc.scalar.activation(t2[:], b2[:], AF.Identity, scale=0.25)
                t3 = tl("t3"); nc.scalar.activation(t3[:], dd[:], AF.Identity, scale=0.5)
                t4 = tl("t4"); nc.scalar.activation(t4[:], e2[:], AF.Identity, scale=0.25)
                t5 = tl("t5"); nc.scalar.activation(t5[:], dd[:], AF.Identity, scale=0.25)
                ds = [dpool.tile([P, CI_T, nq, WP], BF, tag=dtag,
                                 name=f"D_{nm}_{k}") for k in range(KF)]
                nc.vector.tensor_tensor(ds[1][:], p_[:], t1[:], OP.subtract)
                nc.vector.tensor_tensor(ds[2][:], q2[:], t2[:], OP.add)
                nc.vector.tensor_tensor(ds[3][:], c_[:], t3[:], OP.subtract)
                nc.vector.tensor_tensor(ds[4][:], c_[:], t3[:], OP.add)
                nc.vector.tensor_tensor(ds[0][:], c_[:], t4[:], OP.add)
                nc.vector.tensor_tensor(ds[5][:], t5[:], f2[:], OP.subtract)
                return ds

            def mm_group(ms, dsrc, ot, it_outer):
                if it_outer:
                    order = [(it, k, kw) for it in range(CI_T)
                             for k in range(KF) for kw in range(3)]
                else:
                    order = [(it, k, kw) for k in range(KF)
                             for it in range(CI_T) for kw in range(3)]
                for (it, k, kw) in order:
                    dv, q0 = dsrc(it, k)
                    rhs = dv[:, it, q0:q0 + HQ, kw:kw + W]
                    o = woff(ot, it, k, kw)
                    nc.tensor.matmul(ms[k][:], wt[:, o:o + P], rhs,
                                     start=(it == 0 and kw == 0),
                                     stop=(it == CI_T - 1 and kw == 2))

            def out_transform(ms, n, ot, h):
                bias = bt[:, ot:ot + 1]
                nm = f"{n}_{ot}_{h}"

                def oc(r, dt=BF):
                    return ocpool.tile([P, NH], dt, tag=f"oc_{r}",
                                       name=f"{r}_{nm}")
                m0c = oc("m0c")
                nc.scalar.activation(m0c[:], ms[0][:], AF.Identity)
                m1c = oc("m1c", F32)
                nc.scalar.activation(m1c[:], ms[1][:], AF.Identity, bias=bias)
                m5c = oc("m5c")
                nc.scalar.activation(m5c[:], ms[5][:], AF.Identity)
                s_ = oc("s")
                nc.vector.tensor_tensor(s_[:], m1c[:], ms[2][:], OP.add)
                t_ = oc("t")
                nc.vector.tensor_tensor(t_[:], m1c[:], ms[2][:], OP.subtract)
                # TT may read only one PSUM operand: stage M3 in SBUF fp32
                m3c = oc("m3c", F32)
                nc.scalar.activation(m3c[:], ms[3][:], AF.Identity)
                u_ = oc("u")
                nc.vector.tensor_tensor(u_[:], m3c[:], ms[4][:], OP.add)
                v_ = oc("v")
                nc.vector.tensor_tensor(v_[:], m3c[:], ms[4][:], OP.subtract)
                w5 = oc("w5")
                nc.vector.tensor_scalar_mul(w5[:], v_[:], 0.5)
                z2 = oc("z2")
                nc.vector.tensor_scalar_mul(z2[:], u_[:], 0.25)
                h1 = oc("h1")
                nc.vector.tensor_scalar_mul(h1[:], v_[:], 0.125)
                o0a = oc("o0a")
                nc.gpsimd.tensor_tensor(o0a[:], m0c[:], s_[:], OP.add)
                o3a = oc("o3a")
                nc.gpsimd.tensor_tensor(o3a[:], t_[:], h1[:], OP.add)
                ob = opool.tile([P, HQ, 4, W], BF, tag="ob", name=f"ob_{nm}")
                nc.vector.tensor_tensor(ob[:, :, 0, :], o0a[:], u_[:], OP.add)
                nc.gpsimd.tensor_tensor(ob[:, :, 1, :], t_[:], w5[:], OP.add)
                nc.vector.tensor_tensor(ob[:, :, 2, :], s_[:], z2[:], OP.add)
                nc.vector.tensor_tensor(ob[:, :, 3, :], o3a[:], m5c[:], OP.add)
                r0 = 28 * h
                nc.sync.dma_start(
                    out=out_v[n, ot, :, r0 * W:(r0 + 28) * W], in_=ob[:])

            # --- image 0: per-half transforms so matmuls start early ---
            d0 = {}
            d0[0] = transform(pt[0], HQ, d0pool, "d0", "i0_h0", use_pool=False)
            d0[1] = transform(pt[1], HQ, d0pool, "d0", "i0_h1", use_pool=True)
            dfull = {}
            dfull[1] = transform(fulls[1], NQ, dfpool, "df", "i1")

            def run_groups(n):
                for ot in range(CO_T):
                    for hh in range(2):
                        ms = [pspool.tile([P, NH], F32, tag="m",
                                          name=f"m_{n}_{ot}_{hh}_{k}")
                              for k in range(KF)]
                        if n == 0:
                            dsrc = lambda it, k, _h=hh: (d0[_h][k], 0)
                        else:
                            dsrc = lambda it, k, _n=n, _h=hh: (
                                dfull[_n][k], HQ * _h)
                        mm_group(ms, dsrc, ot,
                                 it_outer=(n == 0 and ot == 0 and hh == 0))
                        out_transform(ms, n, ot, hh)

            run_groups(0)
            dfull[2] = transform(fulls[2], NQ, dfpool, "df", "i2")
            run_groups(1)
            dfull[3] = transform(fulls[3], NQ, dfpool, "df", "i3")
            run_groups(2)
            run_groups(3)
    nc.finalize()
    return nc


def _prep_inputs(ip, weight, bias):
    bf16 = ml_dtypes.bfloat16
    ipp = np.zeros((ip.shape[0], CIN, HP, WP), dtype=bf16)
    ipp[:, :, 1:57, 1:57] = ip.astype(bf16)
    # Winograd F(4,3) weight transform along kh, points {0,+-1,+-1/2,inf}
    G = np.array([[4, 0, 0],
                  [2 / 3, 2 / 3, 2 / 3],
                  [2 / 3, -2 / 3, 2 / 3],
                  [-8 / 3, -4 / 3, -2 / 3],
                  [-8 / 3, 4 / 3, -2 / 3],
                  [0, 0, 1]], np.float64)
    U = np.einsum('ka,ocab->kocb', G, weight.astype(np.float64))
    g = U.transpose(2, 0, 3, 1)                  # (ci, k, kw, co)
    g = (g.reshape(CI_T, P, KF, 3, CO_T, P)      # (it, ci_p, k, kw, ot, co_p)
          .transpose(1, 4, 0, 2, 3, 5)           # (ci_p, ot, it, k, kw, co_p)
          .reshape(P, CO_T * CI_T * KF * 3 * P))
    wT = np.ascontiguousarray(g).astype(bf16)
    bT = np.ascontiguousarray(np.asarray(bias, np.float32).reshape(CO_T, P).T)
    return ipp, wT, bT


def kernel(ip, weight, bias, _trace=False, _trace_kwargs=None):
    ip = np.asarray(ip, dtype=np.float32)
    weight = np.asarray(weight, dtype=np.float32)
    bias = np.asarray(bias, dtype=np.float32)

    if "nc" not in _cached:
        _cached["nc"] = _build_nc()
    nc = _cached["nc"]

    ipp, wT, bT = _prep_inputs(ip, weight, bias)
    in_maps = [
        {"ip": ipp[i * NIMG:(i + 1) * NIMG], "weight": wT, "bias": bT}
        for i in range(N_CORES)
    ]
    res = run_bass_kernel_spmd(
        nc, in_maps, core_ids=list(range(N_CORES)),
        trace=_trace, **(_trace_kwargs or {}),
    )
    out = np.concatenate([r["out"] for r in res.results],
                         axis=0).astype(np.float32)
    if _trace:
        return out, res
    return out


# revision 6
# speedup vs baseline: 1.1889x; 1.1889x over previous
"""Conv2d 3x3 via 1-D Winograd F(4,3) along the kh (row) axis.

Half-integer interpolation points {0, +-1, +-1/2, inf} keep every
transform constant an exact power of two in bf16:
  B^T rows (input):  D0 = .25(d0-d2) + (d4-d2)
                     D1 = (d3+d4) - .25(d1+d2)
                     D2 = (d4-d3) + .25(d1-d2)
                     D3 = (d4-d2) - .5(d1-d3)
                     D4 = (d4-d2) + .5(d1-d3)
                     D5 = .25(d1-d3) + (d5-d3)
  A^T rows (output): o0 = m0+m1+m2+m3+m4        o1 = (m1-m2) + .5(m3-m4)
                     o2 = (m1+m2) + .25(m3+m4)  o3 = (m1-m2) + .125(m3-m4) + m5
6 multiplies per 4x1 outputs instead of 12: a 2x TensorEngine FLOP cut
vs direct conv (vs 1.5x for F(2,3)).

Per core (4 images): rows pre-transformed into D_k (k=0..5, quads
q=0..13), weights host-transformed into U_k[ci,kw,co], kw handled as 3
shifted matmul taps.  Per (image, co_tile, half of 7 quads): 6 PSUM
banks M_k each accumulate 6 matmuls (2 ci-tiles x 3 kw, K=128, N=392).
Engine split tuned from HW traces: Pool runs 6 of the 8 input-helper
sums, ACT runs the dyadic input scalings + all PSUM->SBUF bf16 copies,
DVE runs the remaining sums at 2x bf16 mode with the scaled A^T taps
fused into single AFFINE_THEN_ADD custom-DVE ops.  Output stored bf16
and widened to fp32 on the host.
"""

import numpy as np
import ml_dtypes

import concourse.bass as bass
import concourse.mybir as mybir
from concourse import bacc
from concourse.tile import TileContext
from concourse.bass_utils import run_bass_kernel_spmd

P = 128
N_CORES = 8
NIMG = 4
CIN = 256
COUT = 256
H = W = 56
HP = WP = 58
CI_T = 2
CO_T = 2
KF = 6                     # winograd row taps
NQ = 14                    # quad rows per image
HQ = 7                     # quads per half
NH = HQ * W                # matmul N per half = 392

_cached = {}


def _build_nc():
    nc = bacc.Bacc("TRN2", target_bir_lowering=False, debug=False,
                   num_devices=N_CORES)

    ip_h = nc.declare_dram_parameter("ip", [NIMG, CIN, HP, WP],
                                     mybir.dt.bfloat16, isOutput=False)
    w_h = nc.declare_dram_parameter("weight", [P, CO_T * CI_T * KF * 3 * P],
                                    mybir.dt.bfloat16, isOutput=False)
    b_h = nc.declare_dram_parameter("bias", [P, CO_T],
                                    mybir.dt.float32, isOutput=False)
    out_h = nc.declare_dram_parameter("out", [NIMG, COUT, H, W],
                                      mybir.dt.bfloat16, isOutput=True)

    ip_v = ip_h.ap().rearrange("n (t p) h w -> n t p h w", p=P)
    w_v = w_h.ap()
    out_v = out_h.ap().rearrange("n (t p) h w -> n t p (h w)", p=P)

    AF = mybir.ActivationFunctionType
    OP = mybir.AluOpType
    BF = mybir.dt.bfloat16
    F32 = mybir.dt.float32

    def woff(ot, it, k, kw):
        return (((ot * CI_T + it) * KF + k) * 3 + kw) * P

    with TileContext(nc) as tc:
        with (
            tc.tile_pool(name="const", bufs=1) as cpool,
            tc.tile_pool(name="pad0", bufs=4) as ppool,
            tc.tile_pool(name="padf", bufs=4) as pfpool,
            tc.tile_pool(name="d0", bufs=24) as d0pool,
            tc.tile_pool(name="df", bufs=24) as dfpool,
            tc.tile_pool(name="tmp", bufs=16) as tpool,
            tc.tile_pool(name="oc", bufs=3) as ocpool,
            tc.tile_pool(name="outs", bufs=4) as opool,
            tc.tile_pool(name="psum", bufs=8, space="PSUM") as pspool,
        ):
            wt = cpool.tile([P, CO_T * CI_T * KF * 3 * P], BF)
            bt = cpool.tile([P, CO_T], F32)

            # --- DMA staging (single HWDGE queue is FIFO: order by first
            # consumption so the startup ramp is short) ---
            WB = KF * 3 * P            # one (ot, it) weight block width
            nc.sync.dma_start(out=wt[:, 0:WB], in_=w_v[:, 0:WB])  # ot0 it0
            pt = [[None, None], [None, None]]   # [it][h] image-0 half tiles
            pt[0][0] = ppool.tile([P, 32, WP], BF, tag="p0", name="pt00")
            nc.sync.dma_start(out=pt[0][0][:, 0:30, :], in_=ip_v[0, 0, :, 0:30])
            nc.sync.dma_start(out=wt[:, WB:2 * WB],
                              in_=w_v[:, WB:2 * WB])              # ot0 it1
            pt[1][0] = ppool.tile([P, 32, WP], BF, tag="p0", name="pt10")
            nc.sync.dma_start(out=pt[1][0][:, 0:30, :], in_=ip_v[0, 1, :, 0:30])
            nc.sync.dma_start(out=bt[:], in_=b_h.ap())
            pt[0][1] = ppool.tile([P, 32, WP], BF, tag="p0", name="pt01")
            nc.sync.dma_start(out=pt[0][1][:, 0:30, :], in_=ip_v[0, 0, :, 28:58])
            pt[1][1] = ppool.tile([P, 32, WP], BF, tag="p0", name="pt11")
            nc.sync.dma_start(out=pt[1][1][:, 0:30, :], in_=ip_v[0, 1, :, 28:58])
            nc.sync.dma_start(out=wt[:, 2 * WB:], in_=w_v[:, 2 * WB:])  # ot1
            fulls = [[None] * CI_T for _ in range(NIMG)]
            for n in range(1, NIMG):
                for it in range(CI_T):
                    pf = pfpool.tile([P, 60, WP], BF, tag="pf",
                                     name=f"pf_{n}_{it}")
                    nc.sync.dma_start(out=pf[:, 0:HP, :], in_=ip_v[n, it])
                    fulls[n][it] = pf

            def transform(ptile, nq, dpool, dtag, nm, use_pool=True):
                """Row transform: ptile [P, 4*(nq+1), WP] holding rows
                4q+a; returns 6 D_k tiles [P, nq, WP]."""
                v = ptile.rearrange("p (q a) c -> p q a c", a=4)
                d = [v[:, 0:nq, a, :] for a in range(4)]
                d4 = v[:, 1:nq + 1, 0, :]
                d5 = v[:, 1:nq + 1, 1, :]

                def tl(r):
                    return tpool.tile([P, nq, WP], BF, tag="tt",
                                      name=f"t_{nm}_{r}")
                # slow Pool ops first so they overlap the DVE/ACT chain;
                # image-0's first transforms stay off Pool (startup path)
                eng = nc.gpsimd if use_pool else nc.vector
                c_ = tl("c"); eng.tensor_tensor(c_[:], d4, d[2], OP.subtract)
                dd = tl("dd"); eng.tensor_tensor(dd[:], d[1], d[3], OP.subtract)
                p_ = tl("p"); eng.tensor_tensor(p_[:], d[3], d4, OP.add)
                q2 = tl("q"); eng.tensor_tensor(q2[:], d4, d[3], OP.subtract)
                e2 = tl("e"); eng.tensor_tensor(e2[:], d[0], d[2], OP.subtract)
                f2 = tl("f"); eng.tensor_tensor(f2[:], d5, d[3], OP.subtract)
                a2 = tl("a"); nc.vector.tensor_tensor(a2[:], d[1], d[2], OP.add)
                b2 = tl("b"); nc.vector.tensor_tensor(b2[:], d[1], d[2], OP.subtract)
                t1 = tl("t1"); nc.scalar.activation(t1[:], a2[:], AF.Identity, scale=0.25)
                t2 = tl("t2"); nc.scalar.activation(t2[:], b2[:], AF.Identity, scale=0.25)
                t3 = tl("t3"); nc.scalar.activation(t3[:], dd[:], AF.Identity, scale=0.5)
                t4 = tl("t4"); nc.scalar.activation(t4[:], e2[:], AF.Identity, scale=0.25)
                t5 = tl("t5"); nc.scalar.activation(t5[:], dd[:], AF.Identity, scale=0.25)
                ds = [dpool.tile([P, nq, WP], BF, tag=dtag,
                                 name=f"D_{nm}_{k}") for k in range(KF)]
                nc.vector.tensor_tensor(ds[1][:], p_[:], t1[:], OP.subtract)
                nc.vector.tensor_tensor(ds[2][:], q2[:], t2[:], OP.add)
                nc.vector.tensor_tensor(ds[3][:], c_[:], t3[:], OP.subtract)
                nc.vector.tensor_tensor(ds[4][:], c_[:], t3[:], OP.add)
                nc.vector.tensor_tensor(ds[0][:], c_[:], t4[:], OP.add)
                nc.vector.tensor_tensor(ds[5][:], t5[:], f2[:], OP.add)
                return ds

            def mm_group(ms, dsrc, ot, it_outer):
                if it_outer:
                    order = [(it, k, kw) for it in range(CI_T)
                             for k in range(KF) for kw in range(3)]
                else:
                    order = [(it, k, kw) for k in range(KF)
                             for it in range(CI_T) for kw in range(3)]
                for (it, k, kw) in order:
                    dv, q0 = dsrc(it, k)
                    rhs = dv[:, q0:q0 + HQ, kw:kw + W]
                    o = woff(ot, it, k, kw)
                    nc.tensor.matmul(ms[k][:], wt[:, o:o + P], rhs,
                                     start=(it == 0 and kw == 0),
                                     stop=(it == CI_T - 1 and kw == 2))

            def out_transform(ms, n, ot, h):
                bias = bt[:, ot:ot + 1]
                nm = f"{n}_{ot}_{h}"

                def oc(r, dt=BF):
                    return ocpool.tile([P, NH], dt, tag=f"oc_{r}",
                                       name=f"{r}_{nm}")
                # PSUM -> SBUF bf16 copies on ACT (bias folded into m1)
                mc = []
                for k in range(KF):
                    m = oc(f"m{k}c")
                    if k == 1:
                        nc.scalar.activation(m[:], ms[k][:], AF.Identity,
                                             bias=bias)
                    else:
                        nc.scalar.activation(m[:], ms[k][:], AF.Identity)
                    mc.append(m)
                s_ = oc("s")
                nc.vector.tensor_tensor(s_[:], mc[1][:], mc[2][:], OP.add)
                t_ = oc("t")
                nc.vector.tensor_tensor(t_[:], mc[1][:], mc[2][:], OP.subtract)
                u_ = oc("u")
                nc.vector.tensor_tensor(u_[:], mc[3][:], mc[4][:], OP.add)
                v_ = oc("v")
                nc.vector.tensor_tensor(v_[:], mc[3][:], mc[4][:], OP.subtract)
                o0a = oc("o0a")
                nc.vector.tensor_tensor(o0a[:], mc[0][:], s_[:], OP.add)
                o3a = oc("o3a")
                nc.vector.affine_then_add(o3a[:], v_[:], t_[:], 0.125, 0.0)
                ob = opool.tile([P, HQ, 4, W], BF, tag="ob", name=f"ob_{nm}")
                nc.vector.tensor_tensor(ob[:, :, 0, :], o0a[:], u_[:], OP.add)
                nc.vector.affine_then_add(ob[:, :, 1, :], v_[:], t_[:], 0.5, 0.0)
                nc.vector.affine_then_add(ob[:, :, 2, :], u_[:], s_[:], 0.25, 0.0)
                nc.vector.tensor_tensor(ob[:, :, 3, :], o3a[:], mc[5][:], OP.add)
                r0 = 28 * h
                nc.sync.dma_start(
                    out=out_v[n, ot, :, r0 * W:(r0 + 28) * W], in_=ob[:])

            # --- image 0: per-half transforms so matmuls start early ---
            d0 = {}
            for hh in range(2):
                for it in range(CI_T):
                    d0[it, hh] = transform(pt[it][hh], HQ, d0pool, "d0",
                                           f"i0_{it}_{hh}", use_pool=(hh == 1))
            dfull = {}
            for it in range(CI_T):
                dfull[1, it] = transform(fulls[1][it], NQ, dfpool, "df",
                                         f"i1_{it}")

            def run_groups(n):
                for ot in range(CO_T):
                    for hh in range(2):
                        ms = [pspool.tile([P, NH], F32, tag="m",
                                          name=f"m_{n}_{ot}_{hh}_{k}")
                              for k in range(KF)]
                        if n == 0:
                            dsrc = lambda it, k, _h=hh: (d0[it, _h][k], 0)
                        else:
                            dsrc = lambda it, k, _n=n, _h=hh: (
                                dfull[_n, it][k], HQ * _h)
                        mm_group(ms, dsrc, ot,
                                 it_outer=(n == 0 and ot == 0 and hh == 0))
                        out_transform(ms, n, ot, hh)

            run_groups(0)
            for it in range(CI_T):
                dfull[2, it] = transform(fulls[2][it], NQ, dfpool, "df",
                                         f"i2_{it}")
            run_groups(1)
            for it in range(CI_T):
                dfull[3, it] = transform(fulls[3][it], NQ, dfpool, "df",
                                         f"i3_{it}")
            run_groups(2)
            run_groups(3)
    nc.finalize()
    return nc


def _prep_inputs(ip, weight, bias):
    bf16 = ml_dtypes.bfloat16
    ipp = np.zeros((ip.shape[0], CIN, HP, WP), dtype=bf16)
    ipp[:, :, 1:57, 1:57] = ip.astype(bf16)
    # Winograd F(4,3) weight transform along kh, points {0,+-1,+-1/2,inf}
    G = np.array([[4, 0, 0],
                  [2 / 3, 2 / 3, 2 / 3],
                  [2 / 3, -2 / 3, 2 / 3],
                  [-8 / 3, -4 / 3, -2 / 3],
                  [-8 / 3, 4 / 3, -2 / 3],
                  [0, 0, 1]], np.float64)
    U = np.einsum('ka,ocab->kocb', G, weight.astype(np.float64))
    g = U.transpose(2, 0, 3, 1)                  # (ci, k, kw, co)
    g = (g.reshape(CI_T, P, KF, 3, CO_T, P)      # (it, ci_p, k, kw, ot, co_p)
          .transpose(1, 4, 0, 2, 3, 5)           # (ci_p, ot, it, k, kw, co_p)
          .reshape(P, CO_T * CI_T * KF * 3 * P))
    wT = np.ascontiguousarray(g).astype(bf16)
    bT = np.ascontiguousarray(np.asarray(bias, np.float32).reshape(CO_T, P).T)
    return ipp, wT, bT


def kernel(ip, weight, bias, _trace=False, _trace_kwargs=None):
    ip = np.asarray(ip, dtype=np.float32)
    weight = np.asarray(weight, dtype=np.float32)
    bias = np.asarray(bias, dtype=np.float32)

    if "nc" not in _cached:
        _cached["nc"] = _build_nc()
    nc = _cached["nc"]

    ipp, wT, bT = _prep_inputs(ip, weight, bias)
    in_maps = [
        {"ip": ipp[i * NIMG:(i + 1) * NIMG], "weight": wT, "bias": bT}
        for i in range(N_CORES)
    ]
    res = run_bass_kernel_spmd(
        nc, in_maps, core_ids=list(range(N_CORES)),
        trace=_trace, **(_trace_kwargs or {}),
    )
    out = np.concatenate([r["out"] for r in res.results],
                         axis=0).astype(np.float32)
    if _trace:
        return out, res
    return out
